# revision 23
# baseline (speedup 1.0000x reference)
"""Multi-head attention (B=1, S=2048, H=1024, NH=16) on 8 trn2 NeuronCores.

Sharding: head-parallel. Core c owns heads {2c, 2c+1} (= 128 of the 1024
hidden dims). Each core computes its Q/K/V projection slices, the full
attention for its 2 heads, and a full-width partial of the output
projection (contraction over its 128 context dims). Host sums the 8
partials and adds the (host-folded) biases.

Attention elementwise path (the reference quirk: masked scores are set
to 0 pre-softmax, so masked lanes contribute exp(0)=1):

    E = m*exp(s/8) + (1-m)            (m in {0,1})
      = m*(e0 - 1) + 1,   e0 = exp(s/8)

  * Act engine: e0 = Exp(s_psum / 8) straight out of PSUM -> SBUF bf16
    (the only S^2-sized op on Act, the rate-limiting engine).
  * DVE/Pool alternate per iteration on the fused
    e_hat = (e0 - 1) * m   (one scalar_tensor_tensor; mask stays fp8).
  * The "+1" term: sum_j 1*vaug[j,:] = colsum(Vaug) = C, an i-independent
    vector, injected into each PV PSUM accumulation as a single K=2
    matmul against host-precomputed C split into bf16 hi+lo rows.
    The vaug ones-column accumulates sum_j m*(e0-1), so the denominator
    is o[:,64] + C[64] with C[64] = S = 2048 exact in bf16.

Loop structure: token-panel outer (2 panels of 1024 queries), heads
inner, key-block j innermost. Panel 0's output projection + y DMA ride
panel 1's h0 loop, so the tail holds only panel 1's epilogue. PV is
software-pipelined at depth 2 behind S->exp->stt so PE never stalls on
the Act->DVE chain; V-projection chunks ride (p0,h0)'s loop.

DMA discipline (one 625ns HWDGE slot per dma_start, serialized):
batched loads only - weights packed per-need, x in 1MB quarters
(k first, so K-proj finishes during q's load), V in 4 quarter loads,
mask in per-panel row groups timed to land just ahead of their stt,
O^T via ONE batched xbar dma-transpose per panel, y out in 2 DMAs per
panel. q/k biases are folded into the projection evictions
(tensor_scalar_add on DVE/Pool); epilogue normalize on DVE.

Precision: all matmuls bf16 with fp32 PSUM accumulation. Softmax runs
without max-subtraction: the exponent is (q.k/8) ~ N(0, 0.33^2), so exp
never overflows.
"""

import math

import numpy as np
import ml_dtypes

BF16 = ml_dtypes.bfloat16
FP8 = ml_dtypes.float8_e4m3
S, H, NH, DK = 2048, 1024, 16, 64
NCORES = 8
HPC = NH // NCORES          # heads per core = 2
DPC = HPC * DK              # head dims per core = 128
KC = H // 128               # contraction chunks = 8
TP = 2                      # 1024-wide query token panels
JC = S // 128               # 128-wide key chunks = 16
IC = 1024 // 128            # i-chunks per panel = 8
VA = DK + 1                 # v columns + ones column = 65

_CACHE = {}


def _oslc(ic):
    """o_ps column offset for ic-th 65-wide slice: 7 slices in bank 0,
    the 8th at 512 so no matmul crosses a PSUM bank boundary."""
    b, r = divmod(ic, 7)
    return b * 512 + r * VA


def _build_program():
    """Build + compile the (identical) per-core Bass program."""
    from contextlib import ExitStack

    import concourse.bacc as bacc
    import concourse.tile as tile
    from concourse import mybir

    dt = mybir.dt
    AF = mybir.ActivationFunctionType
    ALU = mybir.AluOpType
    f8 = dt.float8e4

    nc = bacc.Bacc("TRN2", target_bir_lowering=False, debug=False)

    qT_d = nc.dram_tensor("qT", [H, S], dt.bfloat16, kind="ExternalInput").ap()
    kT_d = nc.dram_tensor("kT", [H, S], dt.bfloat16, kind="ExternalInput").ap()
    vT_d = nc.dram_tensor("vT", [H, S], dt.bfloat16, kind="ExternalInput").ap()
    maskT_d = nc.dram_tensor("maskT", [S, S], f8, kind="ExternalInput").ap()
    wk_d = nc.dram_tensor("wk", [128, KC * DPC], dt.bfloat16, kind="ExternalInput").ap()
    wq_d = nc.dram_tensor("wq", [128, KC * DPC], dt.bfloat16, kind="ExternalInput").ap()
    wv_d = nc.dram_tensor("wv", [128, KC * DPC], dt.bfloat16, kind="ExternalInput").ap()
    wo_d = nc.dram_tensor("wo", [DPC, H], dt.bfloat16, kind="ExternalInput").ap()
    bqk_d = nc.dram_tensor("bqk", [128, 2], dt.float32, kind="ExternalInput").ap()
    cv_d = nc.dram_tensor("cvec", [2, HPC * VA], dt.bfloat16, kind="ExternalInput").ap()
    id_d = nc.dram_tensor("ident", [128, 128], dt.bfloat16, kind="ExternalInput").ap()
    yT_d = nc.dram_tensor("yT", [H, S], dt.bfloat16, kind="ExternalOutput").ap()

    with tile.TileContext(nc) as tc, ExitStack() as ctx:
        cp = ctx.enter_context(tc.tile_pool(name="const", bufs=1))
        e_p = ctx.enter_context(tc.tile_pool(name="ex", bufs=4))
        eh_p = ctx.enter_context(tc.tile_pool(name="ehat", bufs=5))
        rc_p = ctx.enter_context(tc.tile_pool(name="recip", bufs=2))
        otp_p = ctx.enter_context(tc.tile_pool(name="otpan", bufs=2))
        vin_p = ctx.enter_context(tc.tile_pool(name="vin", bufs=1))
        yp_p = ctx.enter_context(tc.tile_pool(name="ypan", bufs=1))

        # ---- DMA schedule: wk | xk quarters | wq | xq quarters | rest ----
        wk_sb = cp.tile([128, KC * DPC], dt.bfloat16, tag="wk")
        nc.sync.dma_start(out=wk_sb, in_=wk_d)
        # preload the Exp activation table off the critical path
        warm = cp.tile([1, 2], dt.bfloat16, tag="warm")
        nc.vector.memset(warm, 0.0)
        nc.scalar.activation(warm, warm, AF.Exp)

        xin_q = {}
        with tc.tile_pool(name="xin", bufs=1) as xin_p:
            def x_quarters(pre, x_d):
                for c in range(4):
                    xt = xin_p.tile(
                        [128, 2 * S], dt.bfloat16, tag=f"x{pre}{c}",
                        name=f"x{pre}{c}",
                    )
                    nc.sync.dma_start(
                        out=xt.rearrange("p (a i) -> p a i", a=2),
                        in_=x_d[c * 256 : (c + 1) * 256, :].rearrange(
                            "(a p) i -> p a i", p=128
                        ),
                    )
                    xin_q[pre, c] = xt

            x_quarters("k", kT_d)
            wq_sb = cp.tile([128, KC * DPC], dt.bfloat16, tag="wq")
            nc.sync.dma_start(out=wq_sb, in_=wq_d)
            bqk_sb = cp.tile([128, 2], dt.float32, tag="bqk")
            nc.sync.dma_start(out=bqk_sb, in_=bqk_d)
            cv_sb = cp.tile([2, HPC * VA], dt.bfloat16, tag="cv")
            nc.sync.dma_start(out=cv_sb, in_=cv_d)
            ident = cp.tile([128, 128], dt.bfloat16, tag="ident")
            nc.sync.dma_start(out=ident, in_=id_d)
            x_quarters("q", qT_d)

            # ---- Q/K projections; mms follow the quarter arrivals ----
            qT_sb = cp.tile([128, S], dt.bfloat16, tag="qTs")
            kT_sb = cp.tile([128, S], dt.bfloat16, tag="kTs")
            with tc.tile_pool(name="ps_proj", bufs=1, space="PSUM") as pq:
                psl = {
                    pre: [
                        pq.tile([128, 512], dt.float32, tag=f"p{pre}{p}",
                                name=f"p{pre}{p}")
                        for p in range(4)
                    ]
                    for pre in ("q", "k")
                }
                for pre, w in (("k", wk_sb), ("q", wq_sb)):
                    bcol = bqk_sb[:, 1:2] if pre == "k" else bqk_sb[:, 0:1]
                    dest = kT_sb if pre == "k" else qT_sb

                    def evict(p, pre=pre, bcol=bcol, dest=dest):
                        nc.vector.tensor_scalar_add(
                            dest[:, p * 512 : (p + 1) * 512], psl[pre][p], bcol
                        )

                    for c in range(3):
                        for a in range(2):
                            kk = 2 * c + a
                            for p in range(4):
                                nc.tensor.matmul(
                                    psl[pre][p],
                                    lhsT=w[:, kk * DPC : (kk + 1) * DPC],
                                    rhs=xin_q[pre, c][:, a * S + p * 512 : a * S + (p + 1) * 512],
                                    start=(kk == 0),
                                    stop=False,
                                )
                    # last quarter panel-major so panel 0/1 evict first
                    for p in range(4):
                        for a in range(2):
                            kk = 6 + a
                            nc.tensor.matmul(
                                psl[pre][p],
                                lhsT=w[:, kk * DPC : (kk + 1) * DPC],
                                rhs=xin_q[pre, 3][:, a * S + p * 512 : a * S + (p + 1) * 512],
                                start=False,
                                stop=(kk == KC - 1),
                            )
                        evict(p)

        # ---- post-x DMAs: mask p0 head, v quarters, wv, mask rest ----
        vaug = cp.tile([128, JC * (HPC * VA)], dt.bfloat16, tag="vaug")
        nc.gpsimd.memset(
            vaug.rearrange("p (a v) -> p a v", v=VA)[:, :, DK:VA], 1.0
        )
        ot_pan = None
        oT_full = cp.tile([128, S], dt.bfloat16, tag="oTfull")
        y_pan = {}
        mask_sb = cp.tile([128, JC * S], f8, tag="mask")
        mask3 = mask_sb.rearrange("p (j i) -> p j i", i=S)

        def mask_cols(j0, nj, ph):
            nc.sync.dma_start(
                out=mask3[:, j0 : j0 + nj, ph * 1024 : (ph + 1) * 1024],
                in_=maskT_d[
                    j0 * 128 : (j0 + nj) * 128, ph * 1024 : (ph + 1) * 1024
                ].rearrange("(a p) i -> p a i", p=128),
            )

        vin = []

        def v_quarter(c):
            t_ = vin_p.tile(
                [128, KC * 512], dt.bfloat16, tag=f"vq{c}", name=f"vq{c}"
            )
            nc.sync.dma_start(
                out=t_.rearrange("p (a i) -> p a i", a=KC),
                in_=vT_d[:, c * 512 : (c + 1) * 512].rearrange(
                    "(a p) i -> p a i", p=128
                ),
            )
            vin.append(t_)

        mask_cols(0, 2, 0)
        v_quarter(0)
        wv_sb = cp.tile([128, KC * DPC], dt.bfloat16, tag="wv")
        nc.sync.dma_start(out=wv_sb, in_=wv_d)
        mask_cols(2, 6, 0)
        v_quarter(1)
        mask_cols(8, 8, 0)
        v_quarter(2)
        v_quarter(3)
        wo_sb = cp.tile([128, H], dt.bfloat16, tag="wo")
        nc.sync.dma_start(out=wo_sb, in_=wo_d)
        mask_cols(0, 8, 1)
        mask_cols(8, 8, 1)

        ones2 = cp.tile([2, 128], dt.bfloat16, tag="ones2")
        nc.vector.memset(ones2, 1.0)

        # ---- attention: PSUM = s 2x2 + o 1x2 + y/v 2x1 = 8 banks ----
        with tc.tile_pool(name="ps_s", bufs=2, space="PSUM") as ps_p, \
             tc.tile_pool(name="ps_o", bufs=1, space="PSUM") as po_p, \
             tc.tile_pool(name="ps_v", bufs=2, space="PSUM") as pv_p:

            def v_proj_chunk(t):
                """Token-chunk t of the V projection into vaug."""
                ps = pv_p.tile([128, DPC], dt.float32, tag="vps", name=f"pv{t}")
                c, ts_ = divmod(t, 4)
                for kk in range(KC):
                    nc.tensor.matmul(
                        ps,
                        lhsT=vin[c][:, kk * 512 + ts_ * 128 : kk * 512 + (ts_ + 1) * 128],
                        rhs=wv_sb[:, kk * DPC : (kk + 1) * DPC],
                        start=(kk == 0),
                        stop=(kk == KC - 1),
                    )
                base = t * (HPC * VA)
                for h in range(HPC):
                    nc.vector.tensor_copy(
                        vaug[:, base + h * VA : base + h * VA + DK],
                        ps[:, h * DK : (h + 1) * DK],
                    )

            def pv_mms(h, j, et, o_ps):
                for ic in range(IC):
                    nc.tensor.matmul(
                        o_ps[:, _oslc(ic) : _oslc(ic) + VA],
                        lhsT=et[:, ic * 128 : (ic + 1) * 128],
                        rhs=vaug[:, j * (HPC * VA) + h * VA : j * (HPC * VA) + (h + 1) * VA],
                        start=(j == 0 and ic % 7 == 0),
                        stop=False,
                    )

            def c_inject(h, o_ps):
                """+C (hi+lo rows); last slice per bank carries the stop."""
                for ic in range(IC):
                    nc.tensor.matmul(
                        o_ps[:, _oslc(ic) : _oslc(ic) + VA],
                        lhsT=ones2,
                        rhs=cv_sb[:, h * VA : (h + 1) * VA],
                        start=False,
                        stop=(ic in (6, 7)),
                    )

            import concourse.bass as bass_mod

            def norm_bank(p, h, o_ps, ot_pan, b):
                """Normalize one PSUM bank of o_ps into ot_pan."""
                if True:
                    n_ic = (7, 1)[b]
                    rc = rc_p.tile(
                        [128, 8], dt.float32, tag="rc", name=f"rc{p}_{h}_{b}"
                    )
                    den = bass_mod.AP(
                        tensor=o_ps.tensor,
                        offset=o_ps.offset + b * 512 + DK,
                        ap=[o_ps.ap[0], [VA, n_ic]],
                    )
                    nc.vector.reciprocal(rc[:, :n_ic], den)
                    src_ap = bass_mod.AP(
                        tensor=o_ps.tensor,
                        offset=o_ps.offset + b * 512,
                        ap=[o_ps.ap[0], [VA, n_ic], [1, DK]],
                    )
                    rcb = bass_mod.AP(
                        tensor=rc.tensor,
                        offset=rc.offset,
                        ap=[rc.ap[0], [1, n_ic], [0, DK]],
                    )
                    dst = bass_mod.AP(
                        tensor=ot_pan.tensor,
                        offset=ot_pan.offset + b * 7 * 128 + h * DK,
                        ap=[ot_pan.ap[0], [128, n_ic], [1, DK]],
                    )
                    nc.vector.tensor_mul(dst, src_ap, rcb)

            def o_chunk(p, nn, half, eng, pool=None, tag="vps"):
                """One 512-col y chunk: matmul + eviction into y_pan."""
                y_ps = (pool or pv_p).tile(
                    [128, 512], dt.float32, tag=tag, name=f"y{p}_{nn}_{half}"
                )
                nc.tensor.matmul(
                    y_ps,
                    lhsT=wo_sb[:, nn * 128 : (nn + 1) * 128],
                    rhs=oT_full[:, p * 1024 + half * 512 : p * 1024 + (half + 1) * 512],
                    start=True,
                    stop=True,
                )
                eng.tensor_copy(
                    y_pan[p][:, nn * 1024 + half * 512 : nn * 1024 + (half + 1) * 512],
                    y_ps,
                )

            def y_dma(p, lo, hi):
                """DMA y_pan[p] rows nn in [lo,hi) out to yT."""
                nc.sync.dma_start(
                    out=yT_d[lo * 128 : hi * 128, p * 1024 : (p + 1) * 1024]
                    .rearrange("(a p2) i -> p2 a i", p2=128),
                    in_=y_pan[p][:, lo * 1024 : hi * 1024]
                    .rearrange("p (a i) -> p a i", i=1024),
                )

            # ---- flat 64-iteration pipeline ----
            ot_map = {}

            def emit_s(k):
                p, h, j = k // 32, (k // 16) % 2, k % 16
                hs = h * DK
                s_ps = ps_p.tile(
                    [128, 1024], dt.float32, tag="sps", name=f"s{k}"
                )
                with tc.high_priority():
                    for q in range(2):
                        nc.tensor.matmul(
                            s_ps[:, q * 512 : (q + 1) * 512],
                            lhsT=kT_sb[hs : hs + DK, j * 128 : (j + 1) * 128],
                            rhs=qT_sb[hs : hs + DK,
                                      p * 1024 + q * 512 : p * 1024 + (q + 1) * 512],
                            start=True,
                            stop=True,
                        )
                return s_ps

            def transposes(p):
                # batched xbar transposes, bank-0's 7 chunks first
                nc.sync.dma_start_transpose(
                    out=oT_full[:, p * 1024 : p * 1024 + 896].rearrange(
                        "p2 (b c) -> p2 b c", c=128
                    ),
                    in_=ot_map[p][:, 0:896],
                )
                nc.sync.dma_start_transpose(
                    out=oT_full[:, p * 1024 + 896 : (p + 1) * 1024],
                    in_=ot_map[p][:, 896:1024],
                )

            def norm_banks(p, h, o_ps, ot_pan):
                norm_bank(p, h, o_ps, ot_pan, 0)
                norm_bank(p, h, o_ps, ot_pan, 1)

            o_ps_map = {}
            pend = []

            def drain_one():
                pp, ph, pj, peh = pend.pop(0)
                if (pp, ph) not in o_ps_map:
                    o_ps_map[pp, ph] = po_p.tile(
                        [128, 1024], dt.float32, tag="ops", name=f"ops{pp}{ph}"
                    )
                o_ps = o_ps_map[pp, ph]
                pv_mms(ph, pj, peh, o_ps)
                if pj == JC - 1:
                    c_inject(ph, o_ps)
                    if (pp, ph) != (TP - 1, HPC - 1):
                        norm_banks(pp, ph, o_ps, ot_map[pp])
                        if ph == HPC - 1:
                            transposes(pp)

            s_next = emit_s(0)
            for k in range(64):
                p, h, j = k // 32, (k // 16) % 2, k % 16
                if p not in ot_map:
                    ot_map[p] = otp_p.tile(
                        [128, IC * 128], dt.bfloat16, tag="otp", name=f"otp{p}"
                    )
                    y_pan[p] = yp_p.tile(
                        [128, KC * 1024], dt.bfloat16, tag="yp", name=f"ypan{p}"
                    )
                s_ps = s_next
                e0 = e_p.tile(
                    [128, 1024], dt.bfloat16, tag="e0", name=f"e0_{k}"
                )
                nc.scalar.activation(e0, s_ps, AF.Exp, scale=1.0 / math.sqrt(DK))
                eh = eh_p.tile(
                    [128, 1024], dt.bfloat16, tag="eh", name=f"eh{k}"
                )
                eng = nc.vector if (k % 2 == 0 and k != 62) or k == 63 else nc.gpsimd
                eng.scalar_tensor_tensor(
                    eh, e0, 1.0,
                    mask_sb[:, j * S + p * 1024 : j * S + (p + 1) * 1024],
                    ALU.subtract, ALU.mult,
                )
                # next S ahead of PV/side work so Act is never starved
                if k + 1 < 64:
                    s_next = emit_s(k + 1)
                # side work riding this iteration
                if p == 0 and h == 0 and j >= 1:
                    v_proj_chunk(j - 1)
                if p == 0 and h == 1 and j == 0:
                    v_proj_chunk(JC - 1)
                if p == 1 and 36 <= k < 52:
                    # panel-0 output projection, 1 chunk/iter (evictions
                    # must avoid Pool: GPSIMD cannot access PSUM)
                    o_chunk(0, (k - 36) // 2, (k - 36) % 2, nc.vector)
                    if k == 43:
                        y_dma(0, 0, 4)
                    elif k == 51:
                        y_dma(0, 4, 8)
                # depth-4 software pipeline for PV
                if len(pend) == 4:
                    drain_one()
                pend.append((p, h, j, eh))
            while pend:
                drain_one()

            # ---- tail: panel-1 epilogue with PE transposes (PE and
            # all engines idle here; skips the 3us DMA-xbar latency) ----
            def y_dma_cols(p, half, lo, hi):
                nc.sync.dma_start(
                    out=yT_d[lo * 128 : hi * 128,
                             p * 1024 + half * 512 : p * 1024 + (half + 1) * 512]
                    .rearrange("(a p2) i -> p2 a i", p2=128),
                    in_=y_pan[p].rearrange("p (a i) -> p a i", i=1024)[
                        :, lo:hi, half * 512 : (half + 1) * 512
                    ],
                )

            # keep PE's p-state hot while the last stt/PV/norm chain runs
            for f in range(10):
                warm_ps = ps_p.tile(
                    [128, 512], dt.float32, tag="sps", name=f"warm{f}"
                )
                nc.tensor.matmul(
                    warm_ps, lhsT=wo_sb[:, 0:128], rhs=oT_full[:, 0:512],
                    start=True, stop=True,
                )

            o_ps = o_ps_map[TP - 1, HPC - 1]
            ot1 = ot_map[TP - 1]
            rr = (nc.vector, nc.scalar)

            def pe_transpose(lic):
                tp = ps_p.tile(
                    [128, 128], dt.bfloat16, tag="sps", name=f"tp{lic}"
                )
                nc.tensor.transpose(tp, ot1[:, lic * 128 : (lic + 1) * 128], ident)
                eng = rr[lic % 2]
                dst = oT_full[:, 1024 + lic * 128 : 1024 + (lic + 1) * 128]
                if eng is nc.scalar:
                    nc.scalar.activation(dst, tp, AF.Copy)
                else:
                    eng.tensor_copy(dst, tp)

            def tail_chunk(ck, half):
                nn = ck % 8
                eng = rr[ck % 2]
                pool, tag = (pv_p, "vps") if ck % 2 == 0 else (ps_p, "sps")
                if eng is nc.scalar:
                    y_ps = pool.tile(
                        [128, 512], dt.float32, tag=tag, name=f"y1_{ck}"
                    )
                    nc.tensor.matmul(
                        y_ps,
                        lhsT=wo_sb[:, nn * 128 : (nn + 1) * 128],
                        rhs=oT_full[:, 1024 + half * 512 : 1024 + (half + 1) * 512],
                        start=True,
                        stop=True,
                    )
                    nc.scalar.activation(
                        y_pan[1][:, nn * 1024 + half * 512 :
                                 nn * 1024 + (half + 1) * 512],
                        y_ps, AF.Copy,
                    )
                else:
                    o_chunk(1, nn, half, eng, pool=pool, tag=tag)

            norm_bank(TP - 1, HPC - 1, o_ps, ot1, 0)
            for lic in range(7):
                pe_transpose(lic)
            for ck in range(8):
                tail_chunk(ck, 0)
                if ck == 3:
                    y_dma_cols(1, 0, 0, 4)
            y_dma_cols(1, 0, 4, 8)
            norm_bank(TP - 1, HPC - 1, o_ps, ot1, 1)
            pe_transpose(7)
            for ck in range(8, 16):
                tail_chunk(ck, 1)
                if ck == 11:
                    y_dma_cols(1, 1, 0, 4)
            y_dma_cols(1, 1, 4, 8)

    nc.compile()
    return nc


def get_program():
    if "nc" not in _CACHE:
        _CACHE["nc"] = _build_program()
    return _CACHE["nc"]


def _wshuf(wT):
    """[1024 k, 128 n] -> [128 p, KC*128] with chunk kk at cols kk*128."""
    return np.ascontiguousarray(
        wT.reshape(KC, 128, DPC).transpose(1, 0, 2).reshape(128, KC * DPC)
    ).astype(BF16)


def make_in_maps(query, key, value, attention_mask, Wq, bq, Wk, bk, Wv, Wo):
    """Host-side sharding: per-core input dicts."""
    qT = np.ascontiguousarray(np.asarray(query, np.float32)[0].T).astype(BF16)
    kT = np.ascontiguousarray(np.asarray(key, np.float32)[0].T).astype(BF16)
    vT = np.ascontiguousarray(np.asarray(value, np.float32)[0].T).astype(BF16)
    maskT = np.ascontiguousarray(
        np.asarray(attention_mask, np.float32)[0, 0].T
    ).astype(FP8)
    # C = colsum(Vaug) per head = [colsum(value) @ Wv_h.T | S], fp64 on host,
    # split into bf16 hi+lo rows for near-fp32 injection accuracy
    vcol = np.asarray(value, np.float64)[0].sum(axis=0)  # [H]

    in_maps = []
    for c in range(NCORES):
        ns = slice(c * DPC, (c + 1) * DPC)
        cfull = vcol @ np.asarray(Wv, np.float64)[ns].T  # [DPC]
        cvec = np.zeros((2, HPC * VA), np.float64)
        for h in range(HPC):
            cvec[0, h * VA : h * VA + DK] = cfull[h * DK : (h + 1) * DK]
            cvec[0, h * VA + DK] = float(S)
        chi = cvec.astype(BF16)
        clo = (cvec - chi.astype(np.float64)).astype(BF16)
        cboth = np.concatenate([chi[0:1], clo[0:1]], axis=0)
        bqk = np.stack(
            [np.asarray(bq, np.float32)[ns], np.asarray(bk, np.float32)[ns]],
            axis=1,
        )
        in_maps.append(
            {
                "qT": qT,
                "kT": kT,
                "vT": vT,
                "maskT": maskT,
                "wq": _wshuf(np.asarray(Wq, np.float32)[ns].T),
                "wk": _wshuf(np.asarray(Wk, np.float32)[ns].T),
                "wv": _wshuf(np.asarray(Wv, np.float32)[ns].T),
                "wo": np.ascontiguousarray(np.asarray(Wo, np.float32)[:, ns].T).astype(BF16),
                "bqk": np.ascontiguousarray(bqk),
                "cvec": cboth,
                "ident": np.eye(128, dtype=BF16),
            }
        )
    return in_maps


def combine_outputs(results, Wv_bias, Wo, bo):
    """Sum per-core partial yT's (bf16 -> fp32), add host-folded biases."""
    acc = np.zeros((H, S), np.float32)
    for r in results:
        acc += r["yT"].astype(np.float32)
    bias = np.asarray(bo, np.float32) + np.asarray(Wv_bias, np.float32) @ np.asarray(
        Wo, np.float32
    ).T
    return (acc.T + bias[None, :]).astype(np.float32)[None]


def kernel(
    query,
    key,
    value,
    attention_mask,
    Wq,
    bq,
    Wk,
    bk,
    Wv,
    bv,
    Wo,
    bo,
    head,
    hidden_size,
):
    from concourse.bass_utils import run_bass_kernel_spmd

    nc = get_program()
    in_maps = make_in_maps(
        query, key, value, attention_mask, Wq, bq, Wk, bk, Wv, Wo
    )
    res = run_bass_kernel_spmd(nc, in_maps, list(range(NCORES)))
    return combine_outputs(res.results, bv, Wo, bo)
